# revision 8
# baseline (speedup 1.0000x reference)
"""Bidirectional LSTM layer (T=512, B=64, I=H=1024) on 8 trn2 NeuronCores.

Sharding: 2 direction groups x 4-way tensor-parallel over the hidden dim.
Each core owns 256 hidden units of one direction.  Per time step a core:
  1. computes its gate chunk [64, 1024] = h_{t-1} @ w_hh_chunk^T (fp32r
     matmuls, h^T stationary / weights moving) accumulated in PSUM,
  2. adds the precomputed x-projection chunk + bias, applies sigmoid/tanh,
  3. updates its c/h chunk [64, 256],
  4. transposes h to [units, batch] layout on the PE and broadcasts it to its
     3 group peers via remote_dma (XOR-relative routing, so the identical
     SPMD instruction stream works for both groups; a host-side weight-block
     permutation absorbs the XOR slot permutation).
The input projections (half the FLOPs, fully parallel) are computed inside
the same loop in 128-token granules at full PE utilization and live in an
SBUF ring (no DRAM round trip).

Cross-core protocol: arrival semaphores are parity-split (even/odd step)
because successive DMA sends to the same peer are not order-guaranteed.
A "probe" exchange at kernel start reports which logical core landed in
which receive slot; the host validates its assumed logical->physical NC
mapping against it and re-runs with corrected inputs if it ever mismatches.
"""

import contextlib

import numpy as np

import concourse.bacc as bacc
import concourse.mybir as mybir

T, B, I, H = 512, 64, 1024, 1024
HCH = 256  # hidden units per core
GC = 4 * HCH  # gate columns per core (i, f, o, g)
NKH = H // 128  # K-tiles for the hidden matmul
F32 = mybir.dt.float32
F32R = mybir.dt.float32r

LEAD = 4  # x-projection granule lead (in granules; 1 granule = 2 steps)
XPD = 10  # x-projection SBUF ring depth (steps)

# logical core -> physical NC assumption; validated by the probe at runtime.
ASSUMED_PHYS = [0, 1, 2, 3, 4, 5, 6, 7]

N_INIT_DMAS = 7


def build(n_steps=T, lead=LEAD):
    ng = n_steps // 2  # granules
    nc = bacc.Bacc(trn_type="TRN2", num_devices=8)
    ctx = contextlib.ExitStack()

    # ---- DRAM I/O -------------------------------------------------------
    xtg_e = nc.declare_dram_parameter("xtg", [ng, 128, 8 * 128], F32, isOutput=False)
    wih_e = nc.declare_dram_parameter("wih", [128, NKH, GC], F32, isOutput=False)
    whh_e = nc.declare_dram_parameter("whh", [128, NKH, GC], F32, isOutput=False)
    bias_e = nc.declare_dram_parameter("bias", [128, GC], F32, isOutput=False)
    h0t_e = nc.declare_dram_parameter("h0t", [128, NKH, B], F32, isOutput=False)
    c0_e = nc.declare_dram_parameter("c0", [B, HCH], F32, isOutput=False)
    stamp_e = nc.declare_dram_parameter("stamp", [128, 8], F32, isOutput=False)
    iden_e = nc.declare_dram_parameter("iden", [B, B], F32, isOutput=False)

    ys_e = nc.declare_dram_parameter("ys", [n_steps, B, HCH], F32, isOutput=True)
    hT_e = nc.declare_dram_parameter("hT", [B, HCH], F32, isOutput=True)
    cT_e = nc.declare_dram_parameter("cT", [B, HCH], F32, isOutput=True)
    probe_e = nc.declare_dram_parameter("probe", [128, 4, 8], F32, isOutput=True)

    # ---- SBUF -----------------------------------------------------------
    sb = lambda name, shape, dt=F32: ctx.enter_context(nc.sbuf_tensor(name, shape, dt))
    WIH = sb("WIH", [128, NKH, GC], F32R)
    WHH = sb("WHH", [128, NKH, GC], F32R)
    BIAS = sb("BIAS", [128, GC])
    HT2 = [sb(f"HT{p}", [128, NKH, B], F32R) for p in range(2)]  # gathered h^T, by parity
    XT2 = [sb(f"XT{p}", [128, 8, 128], F32R) for p in range(2)]  # x^T granule stationary
    XP = [sb(f"XP{i}", [B, GC]) for i in range(XPD)]  # xproj ring (per step)
    GS = [sb(f"GS{p}", [B, GC]) for p in range(2)]  # gates pre-activation
    AS = [sb(f"AS{p}", [B, GC]) for p in range(2)]  # gates post-activation
    CS = sb("CS", [B, HCH])  # cell state
    TCT = [sb(f"TCT{p}", [B, HCH]) for p in range(2)]  # tanh(c)
    TMA = sb("TMA", [B, HCH])
    TMB = sb("TMB", [B, HCH])
    HS = [sb(f"HS{p}", [B, HCH]) for p in range(2)]  # h chunk
    PROBE = sb("PROBE", [128, 4, 8])
    STAMP = sb("STAMP", [128, 8])
    IDN = sb("IDN", [B, B])
    XSTG = sb("XSTG", [128, GC])

    # ---- PSUM (bank-sized tensors) --------------------------------------
    ps = lambda name: ctx.enter_context(nc.psum_tensor(name, [128, 512], F32))
    pg = [ps("pg0"), ps("pg1")]  # gates accumulator, 2 banks
    pt = [ps("pt0"), ps("pt1")]  # h transposes, by parity
    px = [[ps("px00"), ps("px01")], [ps("px10"), ps("px11")]]  # xproj, by granule parity

    # ---- semaphores ------------------------------------------------------
    s_in = nc.alloc_semaphore("s_in")
    s_xt = [nc.alloc_semaphore(f"s_xt{p}") for p in range(2)]
    s_xmm = nc.alloc_semaphore("s_xmm")
    s_xpc = nc.alloc_semaphore("s_xpc")
    s_xpo = nc.alloc_semaphore("s_xpo")
    s_pe = nc.alloc_semaphore("s_pe")
    s_add = nc.alloc_semaphore("s_add")
    s_act = nc.alloc_semaphore("s_act")
    s_cup = nc.alloc_semaphore("s_cup")
    s_tc = nc.alloc_semaphore("s_tc")
    s_h = nc.alloc_semaphore("s_h")
    s_tp = nc.alloc_semaphore("s_tp")
    s_htc = nc.alloc_semaphore("s_htc")
    s_prep = nc.alloc_semaphore("s_prep")
    s_ys = [nc.alloc_semaphore(f"s_ys{p}") for p in range(2)]
    s_out = nc.alloc_semaphore("s_out")
    s_stc = nc.alloc_semaphore("s_stc")
    s_pls = nc.alloc_semaphore("s_pls")
    s_ls = [nc.alloc_semaphore(f"s_ls{p}") for p in range(2)]
    s_r = [[nc.alloc_semaphore(f"s_r{d}_{p}") for p in range(2)] for d in (1, 2, 3)]
    s_pb = [nc.alloc_semaphore(f"s_pb{d}") for d in (1, 2, 3)]

    def arr_wait(eng, t):
        # wait for step t-1's h broadcasts from all 3 peers (+16 per send)
        if t >= 1:
            for i in range(3):
                eng.wait_ge(s_r[i][(t - 1) % 2], 16 * ((t - 1) // 2 + 1))

    def r32(ap):
        return ap.bitcast(F32R)

    nc.all_core_barrier()

    with nc.Block() as block:

        # ------------------------------------------------ SYNC: DMA in ---
        @block.sync
        def _(sync):
            sync.dma_start(out=WHH[:], in_=whh_e[:].bitcast(F32R)).then_inc(s_in, 16)
            sync.dma_start(out=WIH[:], in_=wih_e[:].bitcast(F32R)).then_inc(s_in, 16)
            sync.dma_start(out=BIAS[:], in_=bias_e[:]).then_inc(s_in, 16)
            sync.dma_start(out=HT2[0][:], in_=h0t_e[:].bitcast(F32R)).then_inc(s_in, 16)
            sync.dma_start(out=CS[:], in_=c0_e[:]).then_inc(s_in, 16)
            sync.dma_start(out=STAMP[:], in_=stamp_e[:]).then_inc(s_in, 16)
            sync.dma_start(out=IDN[:], in_=iden_e[:]).then_inc(s_in, 16)
            def xstg_dma(g):
                sync.wait_ge(s_xpc, 2 * (g + 1))  # granule g's adds done
                sync.dma_start(
                    out=XP[(2 * g + 1) % XPD][:], in_=XSTG[64:128, :]
                ).then_inc(s_xpo, 16)

            for g in range(ng):
                if g >= 2:
                    sync.wait_ge(s_xmm, g - 1)  # granule g-2's matmuls done
                sync.dma_start(
                    out=XT2[g % 2][:], in_=xtg_e[g].rearrange("p (k n) -> p k n", k=8).bitcast(F32R)
                ).then_inc(s_xt[g % 2], 16)
                if g >= 2:
                    xstg_dma(g - 2)
            for g in range(max(0, ng - 2), ng):
                xstg_dma(g)

        # ------------------------------------------------ PE --------------
        @block.tensor
        def _(pe):
            def granule_mms(g, ks, inc=None):
                par = g % 2
                last = None
                for k in ks:
                    for nb in range(2):
                        last = pe.matmul(
                            px[par][nb][0:128, :],
                            lhsT=XT2[g % 2][:, k, :],
                            rhs=WIH[:, k, 512 * nb : 512 * (nb + 1)],
                            start=(k == 0),
                            stop=(k == 7),
                        )
                if inc is not None:
                    last.then_inc(inc, 1)

            # prologue: first `lead` granules
            for g in range(lead):
                pe.wait_ge(s_xt[g % 2], 16 * (g // 2 + 1))
                if g >= 2:
                    pe.wait_ge(s_xpc, 2 * (g - 1))
                granule_mms(g, range(8), inc=s_xmm)

            for t in range(n_steps):
                p, p2 = t % 2, (t + 1) % 2
                # gates: pg free + local slot 0 ready
                if t == 0:
                    pe.wait_ge(s_in, 16 * N_INIT_DMAS)
                else:
                    pe.wait_ge(s_add, 2 * t)
                    pe.wait_ge(s_htc, 2 * t)
                for sl in range(4):
                    if sl >= 1:
                        if t >= 1:
                            pe.wait_ge(s_r[sl - 1][(t - 1) % 2], 16 * ((t - 1) // 2 + 1))
                    for k in (2 * sl, 2 * sl + 1):
                        for nb in range(2):
                            mm = pe.matmul(
                                pg[nb][0:B, :],
                                lhsT=HT2[p][:, k, :],
                                rhs=WHH[:, k, 512 * nb : 512 * (nb + 1)],
                                start=(k == 0),
                                stop=(k == 7),
                            )
                mm.then_inc(s_pe, 1)
                # x-projection granule half
                gi = t // 2 + lead
                if gi < ng:
                    if t % 2 == 0:
                        pe.wait_ge(s_xt[gi % 2], 16 * (gi // 2 + 1))
                        pe.wait_ge(s_xpc, 2 * (gi - 1))
                    granule_mms(
                        gi,
                        range(4 * (t % 2), 4 * (t % 2) + 4),
                        inc=s_xmm if t % 2 == 1 else None,
                    )
                # h transposes into pt[p]
                pe.wait_ge(s_h, t + 1)
                for j in range(2):
                    tr = pe.matmul(
                        pt[p][0:128, 64 * j : 64 * (j + 1)],
                        lhsT=HS[p][:, 128 * j : 128 * (j + 1)],
                        rhs=IDN[:],
                        is_transpose=True,
                        start=True,
                        stop=True,
                    )
                tr.then_inc(s_tp, 1)

        # ------------------------------------------------ DVE -------------
        @block.vector
        def _(vector):
            # probe stamp into probe slot 0
            vector.wait_ge(s_in, 16 * N_INIT_DMAS)
            vector.tensor_copy(PROBE[:, 0, :], STAMP[:]).then_inc(s_stc, 1)
            def granule_epilogue(g):
                vector.wait_ge(s_xmm, g + 1)
                if g >= 1:
                    vector.wait_ge(s_xpo, 16 * g)  # XSTG free (its DMA done)
                for half in range(2):
                    rows = slice(64 * half, 64 * half + 64)
                    for nb in range(2):
                        dst = (
                            XP[(2 * g) % XPD][:, 512 * nb : 512 * (nb + 1)]
                            if half == 0
                            else XSTG[64:128, 512 * nb : 512 * (nb + 1)]
                        )
                        ad = vector.tensor_add(
                            dst,
                            px[g % 2][nb][rows, :],
                            BIAS[rows, 512 * nb : 512 * (nb + 1)],
                        )
                    ad.then_inc(s_xpc, 1)

            # prologue granule epilogues
            for g in range(lead):
                granule_epilogue(g)
            for t in range(n_steps):
                p, p2 = t % 2, (t + 1) % 2
                vector.wait_ge(s_pe, t + 1)
                if t >= 2:
                    vector.wait_ge(s_act, 2 * (t - 1))
                if t % 2 == 0:
                    vector.wait_ge(s_xpc, t + 1)
                else:
                    vector.wait_ge(s_xpo, 16 * (t // 2 + 1))
                for nb in range(2):
                    vector.tensor_add(
                        GS[p][:, 512 * nb : 512 * (nb + 1)],
                        pg[nb][0:B, :],
                        XP[t % XPD][:, 512 * nb : 512 * (nb + 1)],
                    ).then_inc(s_add, 1)
                # c update: c = sig(f)*c + sig(i)*tanh(g); AS layout [i f o g]
                vector.wait_ge(s_act, 2 * (t + 1))
                vector.tensor_mul(TMA[:], AS[p][:, 256:512], CS[:])
                vector.tensor_mul(TMB[:], AS[p][:, 0:256], AS[p][:, 768:1024])
                vector.drain()
                vector.tensor_add(CS[:], TMA[:], TMB[:]).then_inc(s_cup, 1)
                # h = sig(o) * tanh(c)
                vector.wait_ge(s_tc, t + 1)
                if t >= 2:
                    vector.wait_ge(s_ys[t % 2], 16 * (t // 2))
                vector.tensor_mul(HS[p][:], AS[p][:, 512:768], TCT[p][:]).then_inc(
                    s_h, 1
                )
                # copy transposed h into next gather buffer slot 0
                vector.wait_ge(s_tp, t + 1)
                if t >= 2:
                    vector.wait_ge(s_ls[t % 2], 48 * (t // 2))
                for j in range(2):
                    vector.tensor_copy(
                        HT2[p2][:, j, :], pt[p][:, 64 * j : 64 * (j + 1)]
                    ).then_inc(s_htc, 1)
                # x-projection granule epilogue (granule gi finished this step)
                gi = t // 2 + lead
                if t % 2 == 1 and gi < ng:
                    granule_epilogue(gi)

        # ------------------------------------------------ ACT -------------
        @block.scalar
        def _(act):
            SIG = mybir.ActivationFunctionType.Sigmoid
            TANH = mybir.ActivationFunctionType.Tanh
            for t in range(n_steps):
                p = t % 2
                act.wait_ge(s_add, 2 * (t + 1))
                if t >= 2:
                    act.wait_ge(s_h, t - 1)
                act.activation(AS[p][:, 768:1024], GS[p][:, 768:1024], TANH).then_inc(
                    s_act, 1
                )
                act.activation(AS[p][:, 0:768], GS[p][:, 0:768], SIG).then_inc(
                    s_act, 1
                )
                act.wait_ge(s_cup, t + 1)
                act.activation(TCT[p][:], CS[:], TANH).then_inc(s_tc, 1)
                act.wait_ge(s_h, t + 1)
                act.dma_start(out=ys_e[t], in_=HS[p][:]).then_inc(s_ys[t % 2], 16)

        # ------------------------------------------------ GPSIMD ----------
        @block.gpsimd
        def _(gp):
            # probe exchange
            for d in (1, 2, 3):
                gp.remote_dma_broadcast(
                    out_ap=PROBE[:, d, :],
                    in_ap=PROBE[:, 0, :],
                    remote_sem=s_pb[d - 1],
                    local_sem=s_pls,
                    rdests=[(0, d)] * 8,
                ).then_inc(s_prep, 1)
            gp.wait_ge(s_prep, 3)
            gp.wait_ge(s_stc, 1)
            gp.trigger_dma(count=3)
            def emit_preps(t):
                p, p2 = t % 2, (t + 1) % 2
                for d in (1, 2, 3):
                    gp.remote_dma_broadcast(
                        out_ap=HT2[p2][:, 2 * d : 2 * d + 2, :],
                        in_ap=HT2[p2][:, 0:2, :],
                        remote_sem=s_r[d - 1][p],
                        local_sem=s_ls[p],
                        rdests=[(0, d)] * 8,
                    ).then_inc(s_prep, 1)

            emit_preps(0)
            for t in range(n_steps):
                gp.wait_ge(s_prep, 3 * (t + 1) + 3)
                gp.wait_ge(s_htc, 2 * (t + 1))
                arr_wait(gp, t)
                gp.trigger_dma(count=3)
                if t + 1 < n_steps:
                    emit_preps(t + 1)

        # ------------------------------------------------ SYNC: DMA out ---
        @block.sync
        def _(sync):
            arr_wait(sync, n_steps)
            sync.wait_ge(s_h, n_steps)
            sync.wait_ge(s_cup, n_steps)
            sync.wait_ge(s_ys[0], 16 * ((n_steps + 1) // 2))
            sync.wait_ge(s_ys[1], 16 * (n_steps // 2))
            sync.wait_ge(s_ls[0], 48 * ((n_steps + 1) // 2))
            sync.wait_ge(s_ls[1], 48 * (n_steps // 2))
            sync.wait_ge(s_pls, 48)
            for i in range(3):
                sync.wait_ge(s_pb[i], 16)
            sync.dma_start(out=hT_e[:], in_=HS[(n_steps - 1) % 2][:]).then_inc(
                s_out, 16
            )
            sync.dma_start(out=cT_e[:], in_=CS[:]).then_inc(s_out, 16)
            sync.dma_start(out=probe_e[:], in_=PROBE[:]).then_inc(s_out, 16)
            sync.wait_ge(s_out, 48)

    nc.all_core_barrier()
    ctx.close()
    nc.finalize()
    return nc


# ======================================================================
# Host side
# ======================================================================

_GMAP = [0, 1, 3, 2]  # local gate-block order [i, f, o, g] -> torch row blocks


def _gate_rows(c_own):
    return np.concatenate(
        [g * H + c_own * HCH + np.arange(HCH) for g in _GMAP]
    )  # [GC]


def _unit_idx(c_own):
    return np.concatenate(
        [
            HCH * (c_own ^ j) + 128 * half + np.arange(128)
            for j in range(4)
            for half in range(2)
        ]
    )  # [H], k-major


def prep_core_inputs(d, phys, inputs, n_steps=T):
    """Build core d's input dict given logical->physical map `phys`."""
    pd = phys[d]
    c_own, dirb = pd & 3, (pd >> 2) & 1
    sfx = "_b" if dirb else "_f"
    w_ih = np.asarray(inputs["w_ih" + sfx])
    w_hh = np.asarray(inputs["w_hh" + sfx])
    bvec = np.asarray(inputs["b_ih" + sfx]) + np.asarray(inputs["b_hh" + sfx])
    h0 = np.asarray(inputs["h0" + sfx])
    c0 = np.asarray(inputs["c0" + sfx])
    x = np.asarray(inputs["x"])[:n_steps]
    if dirb:
        x = x[::-1]

    rows = _gate_rows(c_own)
    uidx = _unit_idx(c_own)
    ng = n_steps // 2

    wih = np.ascontiguousarray(w_ih[rows].T.reshape(8, 128, GC).transpose(1, 0, 2))
    whh = np.ascontiguousarray(
        w_hh[rows][:, uidx].T.reshape(NKH, 128, GC).transpose(1, 0, 2)
    )
    bias = np.broadcast_to(bvec[rows][None, :], (128, GC)).copy()
    h0t = np.ascontiguousarray(h0[:, uidx].T.reshape(NKH, 128, B).transpose(1, 0, 2))
    c0c = np.ascontiguousarray(c0[:, c_own * HCH : (c_own + 1) * HCH])
    xx = x.reshape(n_steps * B, I)
    xtg = np.ascontiguousarray(
        xx.reshape(ng, 128, 8, 128).transpose(0, 3, 2, 1).reshape(ng, 128, 8 * 128)
    )
    return {
        "xtg": xtg.astype(np.float32),
        "wih": wih.astype(np.float32),
        "whh": whh.astype(np.float32),
        "bias": bias.astype(np.float32),
        "h0t": h0t.astype(np.float32),
        "c0": c0c.astype(np.float32),
        "stamp": np.full((128, 8), float(d), np.float32),
        "iden": np.eye(B, dtype=np.float32),
    }


def decode_probe(results):
    """obs[d][j] = logical id of the core whose chunk landed in slot j."""
    obs = []
    for d in range(8):
        pr = np.asarray(results[d]["probe"]).reshape(128, 4, 8)
        obs.append([int(round(float(pr[0, j, 0]))) for j in range(4)])
    return obs


def probe_consistent(obs, phys):
    for d in range(8):
        for j in range(4):
            s = obs[d][j]
            if not (0 <= s < 8) or phys[s] != phys[d] ^ j:
                return False
    return True


def phys_from_probe(obs):
    """Derive a consistent logical->physical map from observed slots."""
    phys = [None] * 8
    a0 = 0
    phys[a0] = 0
    for j in range(1, 4):
        phys[obs[a0][j]] = j
    rest = [d for d in range(8) if phys[d] is None]
    b0 = rest[0]
    phys[b0] = 4
    for j in range(1, 4):
        phys[obs[b0][j]] = 4 ^ j | 4
    assert all(p is not None for p in phys), f"bad probe: {obs}"
    assert sorted(phys) == list(range(8)), f"inconsistent probe: {obs} -> {phys}"
    return phys


def assemble(results, phys, n_steps=T):
    out = np.zeros((n_steps, B, 2 * H), np.float32)
    hT_f = np.zeros((B, H), np.float32)
    cT_f = np.zeros((B, H), np.float32)
    hT_b = np.zeros((B, H), np.float32)
    cT_b = np.zeros((B, H), np.float32)
    for d in range(8):
        pd = phys[d]
        c_own, dirb = pd & 3, (pd >> 2) & 1
        u = slice(c_own * HCH, (c_own + 1) * HCH)
        ys = np.asarray(results[d]["ys"]).reshape(n_steps, B, HCH)
        if dirb:
            ys = ys[::-1]
        out[:, :, dirb * H + c_own * HCH : dirb * H + (c_own + 1) * HCH] = ys
        hT = np.asarray(results[d]["hT"]).reshape(B, HCH)
        cT = np.asarray(results[d]["cT"]).reshape(B, HCH)
        if dirb:
            hT_b[:, u] = hT
            cT_b[:, u] = cT
        else:
            hT_f[:, u] = hT
            cT_f[:, u] = cT
    return out, hT_f, cT_f, hT_b, cT_b


_BUILT = {}


def _get_module(n_steps=T, lead=LEAD):
    key = (n_steps, lead)
    if key not in _BUILT:
        _BUILT[key] = build(n_steps, lead)
    return _BUILT[key]


def run_on_hw(inputs, phys, n_steps=T):
    from concourse.bass_utils import run_bass_kernel_spmd

    nc = _get_module(n_steps)
    in_maps = [prep_core_inputs(d, phys, inputs, n_steps) for d in range(8)]
    res = run_bass_kernel_spmd(nc, in_maps, core_ids=list(range(8)))
    return res.results


def kernel(**inputs):
    phys = list(ASSUMED_PHYS)
    results = run_on_hw(inputs, phys)
    obs = decode_probe(results)
    if not probe_consistent(obs, phys):
        phys = phys_from_probe(obs)
        results = run_on_hw(inputs, phys)
        obs = decode_probe(results)
        assert probe_consistent(obs, phys), f"probe still inconsistent: {obs}"
    return assemble(results, phys)


# revision 9
# speedup vs baseline: 1.3497x; 1.3497x over previous
"""Bidirectional LSTM layer (T=512, B=64, I=H=1024) on 8 trn2 NeuronCores.

Sharding: 2 direction groups x 4-way tensor-parallel over the hidden dim.
Each core owns 256 hidden units of one direction.  Per time step a core:
  1. computes its gate chunk [64, 1024] = h_{t-1} @ w_hh_chunk^T (fp32r
     matmuls, h^T stationary / weights moving) accumulated in PSUM,
  2. adds the precomputed x-projection chunk + bias, applies sigmoid/tanh,
  3. updates its c/h chunk [64, 256],
  4. transposes h to [units, batch] layout on the PE and broadcasts it to its
     3 group peers via remote_dma (XOR-relative routing, so the identical
     SPMD instruction stream works for both groups; a host-side weight-block
     permutation absorbs the XOR slot permutation).
The input projections (half the FLOPs, fully parallel) are computed inside
the same loop in 128-token granules at full PE utilization and live in an
SBUF ring (no DRAM round trip).

Cross-core protocol: arrival semaphores are parity-split (even/odd step)
because successive DMA sends to the same peer are not order-guaranteed.
A "probe" exchange at kernel start reports which logical core landed in
which receive slot; the host validates its assumed logical->physical NC
mapping against it and re-runs with corrected inputs if it ever mismatches.
"""

import contextlib

import ml_dtypes
import numpy as np

import concourse.bacc as bacc
import concourse.mybir as mybir

T, B, I, H = 512, 64, 1024, 1024
HCH = 256  # hidden units per core
GC = 4 * HCH  # gate columns per core (i, f, o, g)
NKH = H // 128  # K-tiles for the hidden matmul
F32 = mybir.dt.float32
F32R = mybir.dt.float32r
BF16 = mybir.dt.bfloat16

LEAD = 4  # x-projection granule lead (in granules; 1 granule = 2 steps)
XPD = 10  # x-projection SBUF ring depth (steps)

# logical core -> physical NC assumption; validated by the probe at runtime.
ASSUMED_PHYS = [0, 1, 2, 3, 4, 5, 6, 7]

N_INIT_DMAS = 7


def build(n_steps=T, lead=LEAD):
    ng = n_steps // 2  # granules
    nc = bacc.Bacc(trn_type="TRN2", num_devices=8)
    ctx = contextlib.ExitStack()

    # ---- DRAM I/O -------------------------------------------------------
    xtg_e = nc.declare_dram_parameter("xtg", [ng, 128, 8 * 128], BF16, isOutput=False)
    wih_e = nc.declare_dram_parameter("wih", [128, NKH, GC], BF16, isOutput=False)
    whh_e = nc.declare_dram_parameter("whh", [128, NKH, GC], BF16, isOutput=False)
    bias_e = nc.declare_dram_parameter("bias", [128, GC], F32, isOutput=False)
    h0t_e = nc.declare_dram_parameter("h0t", [128, NKH, B], BF16, isOutput=False)
    c0_e = nc.declare_dram_parameter("c0", [B, HCH], F32, isOutput=False)
    stamp_e = nc.declare_dram_parameter("stamp", [128, 8], F32, isOutput=False)
    iden_e = nc.declare_dram_parameter("iden", [B, B], F32, isOutput=False)

    ys_e = nc.declare_dram_parameter("ys", [n_steps, B, HCH], F32, isOutput=True)
    hT_e = nc.declare_dram_parameter("hT", [B, HCH], F32, isOutput=True)
    cT_e = nc.declare_dram_parameter("cT", [B, HCH], F32, isOutput=True)
    probe_e = nc.declare_dram_parameter("probe", [128, 4, 8], F32, isOutput=True)

    # ---- SBUF -----------------------------------------------------------
    sb = lambda name, shape, dt=F32: ctx.enter_context(nc.sbuf_tensor(name, shape, dt))
    WIH = sb("WIH", [128, NKH, GC], BF16)
    WHH = sb("WHH", [128, NKH, GC], BF16)
    BIAS = sb("BIAS", [128, GC])
    HT2 = [sb(f"HT{p}", [128, NKH, B], BF16) for p in range(2)]  # gathered h^T, by parity
    XT2 = [sb(f"XT{p}", [128, 8, 128], BF16) for p in range(2)]  # x^T granule stationary
    XP = [sb(f"XP{i}", [B, GC]) for i in range(XPD)]  # xproj ring (per step)
    GS = [sb(f"GS{p}", [B, GC]) for p in range(2)]  # gates pre-activation
    AS = [sb(f"AS{p}", [B, GC]) for p in range(2)]  # gates post-activation
    CS = sb("CS", [B, HCH])  # cell state
    TCT = [sb(f"TCT{p}", [B, HCH]) for p in range(2)]  # tanh(c)
    TMA = sb("TMA", [B, HCH])
    TMB = sb("TMB", [B, HCH])
    HS = [sb(f"HS{p}", [B, HCH]) for p in range(2)]  # h chunk
    PROBE = sb("PROBE", [128, 4, 8])
    STAMP = sb("STAMP", [128, 8])
    IDN = sb("IDN", [B, B])
    XSTG = sb("XSTG", [128, GC])

    # ---- PSUM (bank-sized tensors) --------------------------------------
    ps = lambda name: ctx.enter_context(nc.psum_tensor(name, [128, 512], F32))
    pg = [ps("pg0"), ps("pg1")]  # gates accumulator, 2 banks
    pt = [ps("pt0"), ps("pt1")]  # h transposes, by parity
    px = [[ps("px00"), ps("px01")], [ps("px10"), ps("px11")]]  # xproj, by granule parity

    # ---- semaphores ------------------------------------------------------
    s_in = nc.alloc_semaphore("s_in")
    s_xt = [nc.alloc_semaphore(f"s_xt{p}") for p in range(2)]
    s_xmm = nc.alloc_semaphore("s_xmm")
    s_xpc = nc.alloc_semaphore("s_xpc")
    s_xpo = nc.alloc_semaphore("s_xpo")
    s_pe = nc.alloc_semaphore("s_pe")
    s_add = nc.alloc_semaphore("s_add")
    s_act = nc.alloc_semaphore("s_act")
    s_cup = nc.alloc_semaphore("s_cup")
    s_tc = nc.alloc_semaphore("s_tc")
    s_h = nc.alloc_semaphore("s_h")
    s_tp = nc.alloc_semaphore("s_tp")
    s_htc = nc.alloc_semaphore("s_htc")
    s_prep = nc.alloc_semaphore("s_prep")
    s_ys = [nc.alloc_semaphore(f"s_ys{p}") for p in range(2)]
    s_out = nc.alloc_semaphore("s_out")
    s_stc = nc.alloc_semaphore("s_stc")
    s_pls = nc.alloc_semaphore("s_pls")
    s_ls = [nc.alloc_semaphore(f"s_ls{p}") for p in range(2)]
    s_r = [[nc.alloc_semaphore(f"s_r{d}_{p}") for p in range(2)] for d in (1, 2, 3)]
    s_pb = [nc.alloc_semaphore(f"s_pb{d}") for d in (1, 2, 3)]

    def arr_wait(eng, t):
        # wait for step t-1's h broadcasts from all 3 peers (+16 per send)
        if t >= 1:
            for i in range(3):
                eng.wait_ge(s_r[i][(t - 1) % 2], 16 * ((t - 1) // 2 + 1))

    def r32(ap):
        return ap.bitcast(F32R)

    nc.all_core_barrier()

    with nc.Block() as block:

        # ------------------------------------------------ SYNC: DMA in ---
        @block.sync
        def _(sync):
            sync.dma_start(out=WHH[:], in_=whh_e[:]).then_inc(s_in, 16)
            sync.dma_start(out=WIH[:], in_=wih_e[:]).then_inc(s_in, 16)
            sync.dma_start(out=BIAS[:], in_=bias_e[:]).then_inc(s_in, 16)
            sync.dma_start(out=HT2[0][:], in_=h0t_e[:]).then_inc(s_in, 16)
            sync.dma_start(out=CS[:], in_=c0_e[:]).then_inc(s_in, 16)
            sync.dma_start(out=STAMP[:], in_=stamp_e[:]).then_inc(s_in, 16)
            sync.dma_start(out=IDN[:], in_=iden_e[:]).then_inc(s_in, 16)
            def xstg_dma(g):
                sync.wait_ge(s_xpc, 2 * (g + 1))  # granule g's adds done
                sync.dma_start(
                    out=XP[(2 * g + 1) % XPD][:], in_=XSTG[64:128, :]
                ).then_inc(s_xpo, 16)

            for g in range(ng):
                if g >= 2:
                    sync.wait_ge(s_xmm, g - 1)  # granule g-2's matmuls done
                sync.dma_start(
                    out=XT2[g % 2][:], in_=xtg_e[g].rearrange("p (k n) -> p k n", k=8)
                ).then_inc(s_xt[g % 2], 16)
                if g >= 2:
                    xstg_dma(g - 2)
            for g in range(max(0, ng - 2), ng):
                xstg_dma(g)

        # ------------------------------------------------ PE --------------
        @block.tensor
        def _(pe):
            def granule_mms(g, ks, inc=None):
                par = g % 2
                last = None
                for k in ks:
                    for nb in range(2):
                        last = pe.matmul(
                            px[par][nb][0:128, :],
                            lhsT=XT2[g % 2][:, k, :],
                            rhs=WIH[:, k, 512 * nb : 512 * (nb + 1)],
                            start=(k == 0),
                            stop=(k == 7),
                        )
                if inc is not None:
                    last.then_inc(inc, 1)

            # prologue: first `lead` granules
            for g in range(lead):
                pe.wait_ge(s_xt[g % 2], 16 * (g // 2 + 1))
                if g >= 2:
                    pe.wait_ge(s_xpc, 2 * (g - 1))
                granule_mms(g, range(8), inc=s_xmm)

            for t in range(n_steps):
                p, p2 = t % 2, (t + 1) % 2
                # gates: pg free + local slot 0 ready
                if t == 0:
                    pe.wait_ge(s_in, 16 * N_INIT_DMAS)
                else:
                    pe.wait_ge(s_add, 2 * t)
                    pe.wait_ge(s_htc, 2 * t)
                for sl in range(4):
                    if sl >= 1:
                        if t >= 1:
                            pe.wait_ge(s_r[sl - 1][(t - 1) % 2], 16 * ((t - 1) // 2 + 1))
                    for k in (2 * sl, 2 * sl + 1):
                        for nb in range(2):
                            mm = pe.matmul(
                                pg[nb][0:B, :],
                                lhsT=HT2[p][:, k, :],
                                rhs=WHH[:, k, 512 * nb : 512 * (nb + 1)],
                                start=(k == 0),
                                stop=(k == 7),
                            )
                mm.then_inc(s_pe, 1)
                # x-projection granule half
                gi = t // 2 + lead
                if gi < ng:
                    if t % 2 == 0:
                        pe.wait_ge(s_xt[gi % 2], 16 * (gi // 2 + 1))
                        pe.wait_ge(s_xpc, 2 * (gi - 1))
                    granule_mms(
                        gi,
                        range(4 * (t % 2), 4 * (t % 2) + 4),
                        inc=s_xmm if t % 2 == 1 else None,
                    )
                # h transposes into pt[p]
                pe.wait_ge(s_h, t + 1)
                for j in range(2):
                    tr = pe.matmul(
                        pt[p][0:128, 64 * j : 64 * (j + 1)],
                        lhsT=HS[p][:, 128 * j : 128 * (j + 1)],
                        rhs=IDN[:],
                        is_transpose=True,
                        start=True,
                        stop=True,
                    )
                tr.then_inc(s_tp, 1)

        # ------------------------------------------------ DVE -------------
        @block.vector
        def _(vector):
            # probe stamp into probe slot 0
            vector.wait_ge(s_in, 16 * N_INIT_DMAS)
            vector.tensor_copy(PROBE[:, 0, :], STAMP[:]).then_inc(s_stc, 1)
            def granule_epilogue(g):
                vector.wait_ge(s_xmm, g + 1)
                if g >= 1:
                    vector.wait_ge(s_xpo, 16 * g)  # XSTG free (its DMA done)
                for half in range(2):
                    rows = slice(64 * half, 64 * half + 64)
                    for nb in range(2):
                        dst = (
                            XP[(2 * g) % XPD][:, 512 * nb : 512 * (nb + 1)]
                            if half == 0
                            else XSTG[64:128, 512 * nb : 512 * (nb + 1)]
                        )
                        ad = vector.tensor_add(
                            dst,
                            px[g % 2][nb][rows, :],
                            BIAS[rows, 512 * nb : 512 * (nb + 1)],
                        )
                    ad.then_inc(s_xpc, 1)

            # prologue granule epilogues
            for g in range(lead):
                granule_epilogue(g)
            for t in range(n_steps):
                p, p2 = t % 2, (t + 1) % 2
                vector.wait_ge(s_pe, t + 1)
                if t >= 2:
                    vector.wait_ge(s_act, 2 * (t - 1))
                if t % 2 == 0:
                    vector.wait_ge(s_xpc, t + 1)
                else:
                    vector.wait_ge(s_xpo, 16 * (t // 2 + 1))
                for nb in range(2):
                    vector.tensor_add(
                        GS[p][:, 512 * nb : 512 * (nb + 1)],
                        pg[nb][0:B, :],
                        XP[t % XPD][:, 512 * nb : 512 * (nb + 1)],
                    ).then_inc(s_add, 1)
                # c update: c = sig(f)*c + sig(i)*tanh(g); AS layout [i f o g]
                vector.wait_ge(s_act, 2 * (t + 1))
                vector.tensor_mul(TMA[:], AS[p][:, 256:512], CS[:])
                vector.tensor_mul(TMB[:], AS[p][:, 0:256], AS[p][:, 768:1024])
                vector.drain()
                vector.tensor_add(CS[:], TMA[:], TMB[:]).then_inc(s_cup, 1)
                # h = sig(o) * tanh(c)
                vector.wait_ge(s_tc, t + 1)
                if t >= 2:
                    vector.wait_ge(s_ys[t % 2], 16 * (t // 2))
                vector.tensor_mul(HS[p][:], AS[p][:, 512:768], TCT[p][:]).then_inc(
                    s_h, 1
                )
                # copy transposed h into next gather buffer slot 0
                vector.wait_ge(s_tp, t + 1)
                if t >= 2:
                    vector.wait_ge(s_ls[t % 2], 48 * (t // 2))
                for j in range(2):
                    vector.tensor_copy(
                        HT2[p2][:, j, :], pt[p][:, 64 * j : 64 * (j + 1)]
                    ).then_inc(s_htc, 1)
                # x-projection granule epilogue (granule gi finished this step)
                gi = t // 2 + lead
                if t % 2 == 1 and gi < ng:
                    granule_epilogue(gi)

        # ------------------------------------------------ ACT -------------
        @block.scalar
        def _(act):
            SIG = mybir.ActivationFunctionType.Sigmoid
            TANH = mybir.ActivationFunctionType.Tanh
            for t in range(n_steps):
                p = t % 2
                act.wait_ge(s_add, 2 * (t + 1))
                if t >= 2:
                    act.wait_ge(s_h, t - 1)
                act.activation(AS[p][:, 768:1024], GS[p][:, 768:1024], TANH).then_inc(
                    s_act, 1
                )
                act.activation(AS[p][:, 0:768], GS[p][:, 0:768], SIG).then_inc(
                    s_act, 1
                )
                act.wait_ge(s_cup, t + 1)
                act.activation(TCT[p][:], CS[:], TANH).then_inc(s_tc, 1)
                act.wait_ge(s_h, t + 1)
                act.dma_start(out=ys_e[t], in_=HS[p][:]).then_inc(s_ys[t % 2], 16)

        # ------------------------------------------------ GPSIMD ----------
        @block.gpsimd
        def _(gp):
            # probe exchange
            for d in (1, 2, 3):
                gp.remote_dma_broadcast(
                    out_ap=PROBE[:, d, :],
                    in_ap=PROBE[:, 0, :],
                    remote_sem=s_pb[d - 1],
                    local_sem=s_pls,
                    rdests=[(0, d)] * 8,
                ).then_inc(s_prep, 1)
            gp.wait_ge(s_prep, 3)
            gp.wait_ge(s_stc, 1)
            gp.trigger_dma(count=3)
            def emit_preps(t):
                p, p2 = t % 2, (t + 1) % 2
                for d in (1, 2, 3):
                    gp.remote_dma_broadcast(
                        out_ap=HT2[p2][:, 2 * d : 2 * d + 2, :],
                        in_ap=HT2[p2][:, 0:2, :],
                        remote_sem=s_r[d - 1][p],
                        local_sem=s_ls[p],
                        rdests=[(0, d)] * 8,
                    ).then_inc(s_prep, 1)

            emit_preps(0)
            for t in range(n_steps):
                gp.wait_ge(s_prep, 3 * (t + 1) + 3)
                gp.wait_ge(s_htc, 2 * (t + 1))
                arr_wait(gp, t)
                gp.trigger_dma(count=3)
                if t + 1 < n_steps:
                    emit_preps(t + 1)

        # ------------------------------------------------ SYNC: DMA out ---
        @block.sync
        def _(sync):
            arr_wait(sync, n_steps)
            sync.wait_ge(s_h, n_steps)
            sync.wait_ge(s_cup, n_steps)
            sync.wait_ge(s_ys[0], 16 * ((n_steps + 1) // 2))
            sync.wait_ge(s_ys[1], 16 * (n_steps // 2))
            sync.wait_ge(s_ls[0], 48 * ((n_steps + 1) // 2))
            sync.wait_ge(s_ls[1], 48 * (n_steps // 2))
            sync.wait_ge(s_pls, 48)
            for i in range(3):
                sync.wait_ge(s_pb[i], 16)
            sync.dma_start(out=hT_e[:], in_=HS[(n_steps - 1) % 2][:]).then_inc(
                s_out, 16
            )
            sync.dma_start(out=cT_e[:], in_=CS[:]).then_inc(s_out, 16)
            sync.dma_start(out=probe_e[:], in_=PROBE[:]).then_inc(s_out, 16)
            sync.wait_ge(s_out, 48)

    nc.all_core_barrier()
    ctx.close()
    nc.finalize()
    return nc


# ======================================================================
# Host side
# ======================================================================

_GMAP = [0, 1, 3, 2]  # local gate-block order [i, f, o, g] -> torch row blocks


def _gate_rows(c_own):
    return np.concatenate(
        [g * H + c_own * HCH + np.arange(HCH) for g in _GMAP]
    )  # [GC]


def _unit_idx(c_own):
    return np.concatenate(
        [
            HCH * (c_own ^ j) + 128 * half + np.arange(128)
            for j in range(4)
            for half in range(2)
        ]
    )  # [H], k-major


def prep_core_inputs(d, phys, inputs, n_steps=T):
    """Build core d's input dict given logical->physical map `phys`."""
    pd = phys[d]
    c_own, dirb = pd & 3, (pd >> 2) & 1
    sfx = "_b" if dirb else "_f"
    w_ih = np.asarray(inputs["w_ih" + sfx])
    w_hh = np.asarray(inputs["w_hh" + sfx])
    bvec = np.asarray(inputs["b_ih" + sfx]) + np.asarray(inputs["b_hh" + sfx])
    h0 = np.asarray(inputs["h0" + sfx])
    c0 = np.asarray(inputs["c0" + sfx])
    x = np.asarray(inputs["x"])[:n_steps]
    if dirb:
        x = x[::-1]

    rows = _gate_rows(c_own)
    uidx = _unit_idx(c_own)
    ng = n_steps // 2

    wih = np.ascontiguousarray(w_ih[rows].T.reshape(8, 128, GC).transpose(1, 0, 2))
    whh = np.ascontiguousarray(
        w_hh[rows][:, uidx].T.reshape(NKH, 128, GC).transpose(1, 0, 2)
    )
    bias = np.broadcast_to(bvec[rows][None, :], (128, GC)).copy()
    h0t = np.ascontiguousarray(h0[:, uidx].T.reshape(NKH, 128, B).transpose(1, 0, 2))
    c0c = np.ascontiguousarray(c0[:, c_own * HCH : (c_own + 1) * HCH])
    xx = x.reshape(n_steps * B, I)
    xtg = np.ascontiguousarray(
        xx.reshape(ng, 128, 8, 128).transpose(0, 3, 2, 1).reshape(ng, 128, 8 * 128)
    )
    return {
        "xtg": xtg.astype(ml_dtypes.bfloat16),
        "wih": wih.astype(ml_dtypes.bfloat16),
        "whh": whh.astype(ml_dtypes.bfloat16),
        "bias": bias.astype(np.float32),
        "h0t": h0t.astype(ml_dtypes.bfloat16),
        "c0": c0c.astype(np.float32),
        "stamp": np.full((128, 8), float(d), np.float32),
        "iden": np.eye(B, dtype=np.float32),
    }


def decode_probe(results):
    """obs[d][j] = logical id of the core whose chunk landed in slot j."""
    obs = []
    for d in range(8):
        pr = np.asarray(results[d]["probe"]).reshape(128, 4, 8)
        obs.append([int(round(float(pr[0, j, 0]))) for j in range(4)])
    return obs


def probe_consistent(obs, phys):
    for d in range(8):
        for j in range(4):
            s = obs[d][j]
            if not (0 <= s < 8) or phys[s] != phys[d] ^ j:
                return False
    return True


def phys_from_probe(obs):
    """Derive a consistent logical->physical map from observed slots."""
    phys = [None] * 8
    a0 = 0
    phys[a0] = 0
    for j in range(1, 4):
        phys[obs[a0][j]] = j
    rest = [d for d in range(8) if phys[d] is None]
    b0 = rest[0]
    phys[b0] = 4
    for j in range(1, 4):
        phys[obs[b0][j]] = 4 ^ j | 4
    assert all(p is not None for p in phys), f"bad probe: {obs}"
    assert sorted(phys) == list(range(8)), f"inconsistent probe: {obs} -> {phys}"
    return phys


def assemble(results, phys, n_steps=T):
    out = np.zeros((n_steps, B, 2 * H), np.float32)
    hT_f = np.zeros((B, H), np.float32)
    cT_f = np.zeros((B, H), np.float32)
    hT_b = np.zeros((B, H), np.float32)
    cT_b = np.zeros((B, H), np.float32)
    for d in range(8):
        pd = phys[d]
        c_own, dirb = pd & 3, (pd >> 2) & 1
        u = slice(c_own * HCH, (c_own + 1) * HCH)
        ys = np.asarray(results[d]["ys"]).reshape(n_steps, B, HCH)
        if dirb:
            ys = ys[::-1]
        out[:, :, dirb * H + c_own * HCH : dirb * H + (c_own + 1) * HCH] = ys
        hT = np.asarray(results[d]["hT"]).reshape(B, HCH)
        cT = np.asarray(results[d]["cT"]).reshape(B, HCH)
        if dirb:
            hT_b[:, u] = hT
            cT_b[:, u] = cT
        else:
            hT_f[:, u] = hT
            cT_f[:, u] = cT
    return out, hT_f, cT_f, hT_b, cT_b


_BUILT = {}


def _get_module(n_steps=T, lead=LEAD):
    key = (n_steps, lead)
    if key not in _BUILT:
        _BUILT[key] = build(n_steps, lead)
    return _BUILT[key]


def run_on_hw(inputs, phys, n_steps=T):
    from concourse.bass_utils import run_bass_kernel_spmd

    nc = _get_module(n_steps)
    in_maps = [prep_core_inputs(d, phys, inputs, n_steps) for d in range(8)]
    res = run_bass_kernel_spmd(nc, in_maps, core_ids=list(range(8)))
    return res.results


def kernel(**inputs):
    phys = list(ASSUMED_PHYS)
    results = run_on_hw(inputs, phys)
    obs = decode_probe(results)
    if not probe_consistent(obs, phys):
        phys = phys_from_probe(obs)
        results = run_on_hw(inputs, phys)
        obs = decode_probe(results)
        assert probe_consistent(obs, phys), f"probe still inconsistent: {obs}"
    return assemble(results, phys)
